# revision 3
# baseline (speedup 1.0000x reference)
"""Single-head attention (batch 8, seq 4096, embed 1024, head 64) on 8 TRN2
NeuronCores, data-parallel over batch (one batch element per core).

v2 design (all matmuls bf16, fp32 PSUM):
  1. x^T via DMA-transpose split per (e-chunk, s-block) so projections start
     after the first 2MB lands rather than all 8MB.
  2. Projections: [Wq|Wv] pass -> qv tiles (Q^T rows 0:64, V^T rows 64:128),
     [Wk|Wk] pass -> kt tiles (K^T duplicated in both partition halves).
     Q^T duplicated to partitions 64:128 of qq tiles via SBUF->SBUF DMA.
     V^T PE-transposed to natural [128k, 65] tiles with a ones column
     (softmax denominator rides the PV matmul).
  3. S^T row-tiled: even k-chunks matmul in PE rows 0:64, odd chunks in rows
     64:128 (tile_position row groups) - the two N=512 matmuls run
     concurrently, halving score-matmul time. Each pair writes one
     [128, 1024] PSUM tile (bank-aligned halves).
  4. exp split across two engines: ScalarE true exp (table spline) and DVE
     Schraudolph (one tensor_scalar: i16 = s*23.0812 + B, bitcast bf16),
     interleaved per chunk-pair; bias tuned for zero mean error.
  5. PV with P^T stationary (FWL weight load) and [V|ones] as the N=65
     moving operand: out[q,h] lands in natural layout with the softmax
     denominator in column 64 - no transpose epilogue. Four 65-wide
     accumulators pack into one PSUM bank, pre-zeroed by a K=1 matmul so
     all PV matmuls accumulate with start=False (avoids the 2KB
     zero-region hazard).
  6. Epilogue: strided reciprocal of the 4 denominator columns (DVE), four
     ScalarE copy-with-scale normalizes, one DMA out per 512-q block.
"""

import numpy as np

import concourse.bass as bass
import concourse.mybir as mybir
import concourse.tile as tile
from concourse import bacc
from concourse.bass_utils import run_bass_kernel_spmd

S = 4096  # sequence length (per core)
E = 1024  # embed dim
H = 64  # head size
B = 8  # batch == number of cores

SB = 1024  # projection s-block
NSB = S // SB
QB = 512  # attention q-block
NQB = S // QB
CH = 128  # k chunk
NCH = S // CH
NPAIR = NCH // 2  # chunk pairs per q-block

f32 = mybir.dt.float32
bf16 = mybir.dt.bfloat16
i16 = mybir.dt.int16
EXP = mybir.ActivationFunctionType.Exp
COPY = mybir.ActivationFunctionType.Copy
MULT = mybir.AluOpType.mult
ADD = mybir.AluOpType.add

_cache = {}

# --- tuning flags -----------------------------------------------------------
ROWTILE = True  # concurrent row-tiled S^T pairs
PV_STAT = True  # P^T-stationary PV (natural output) vs V-stationary
N_DVE = 7  # of every 16 chunk-pairs, how many exp'd by DVE Schraudolph
SCH_A = 128.0 * np.log2(np.e) / 8.0  # 23.0812... (the /8 score scale folded in)
SCH_B = 16256.0 - 7.3  # zero-mean bias; +0.5 if hw truncates on int convert


def _dve_pair(jp):
    # Bresenham-interleave N_DVE of every 16 pairs onto the DVE
    return ((jp * N_DVE) % NPAIR) < N_DVE


def _emit_iteration(nc, tc, ps, pp, consts):
    eye, wqv, wkk, x_d, ones_d, zeros_d, out_d = consts
    EC = E // 128

    qv_tiles = []  # [128, SB]: rows 0:64 Q^T, rows 64:128 V^T
    kt_tiles = []  # [128, SB]: K^T duplicated in both halves
    qq_tiles = []  # [128, SB]: rows 64:128 = Q^T copy (DMA)
    for sb in range(NSB):
        qv_tiles.append(pp.tile([128, SB], bf16, tag=f"qv{sb}", name=f"qv{sb}"))
        kt_tiles.append(pp.tile([128, SB], bf16, tag=f"kt{sb}", name=f"kt{sb}"))
        qq_tiles.append(pp.tile([128, SB], bf16, tag=f"qq{sb}", name=f"qq{sb}"))
    v_tiles = []  # [128, 65] V natural + ones column
    for j in range(NCH):
        v_tiles.append(pp.tile([128, 65], bf16, tag=f"v{j}", name=f"v{j}"))
    zb = pp.tile([1, 384], bf16, tag="zb", name="zb")
    nc.sync.dma_start(out=zb[:], in_=zeros_d[:])
    for j in range(NCH):
        nc.sync.dma_start(out=v_tiles[j][:, 64:65], in_=ones_d[:])

    with (
        tc.tile_pool(name="xt", bufs=1) as xtp,
        tc.tile_pool(name="pt", bufs=4) as ptp,
        tc.tile_pool(name="park", bufs=1) as parkp,
        tc.tile_pool(name="eo", bufs=2) as eop,
    ):
        xt_blk = []
        for c in range(EC):
            xt_c = xtp.tile([128, S], bf16, tag=f"xt{c}", name=f"xt{c}")
            xt_blk.append(xt_c)
        # split transposed loads: s-block-major so early passes unblock first
        for sb in range(NSB):
            for c in range(EC):
                nc.sync.dma_start_transpose(
                    xt_blk[c][:, sb * SB : (sb + 1) * SB],
                    x_d[sb * SB : (sb + 1) * SB, c * 128 : (c + 1) * 128],
                )

        # ---------------- S^T + exp helpers ----------------
        def emit_st_pair(m, jp, pool):
            """Row-tiled score pair for q-block m, chunk pair jp -> exp'd P^T
            pair tile [128, 1024] bf16 (cols 0:512 chunk 2jp, 512:1024 chunk
            2jp+1). Returns the SBUF tile."""
            msb, mo = m // 2, (m % 2) * QB
            j0, j1 = 2 * jp, 2 * jp + 1
            st = ps.tile([128, 1024], f32, tag="st", bufs=3, name=f"st{m}_{jp}")
            k0 = kt_tiles[j0 // 8][0:64, (j0 % 8) * 128 : (j0 % 8 + 1) * 128]
            nc.tensor.matmul(
                st[:, 0:512],
                k0,
                qv_tiles[msb][0:64, mo : mo + QB],
                start=True,
                stop=True,
            )
            if ROWTILE:
                k1 = kt_tiles[j1 // 8][64:128, (j1 % 8) * 128 : (j1 % 8 + 1) * 128]
                nc.tensor.matmul(
                    st[:, 512:1024],
                    k1,
                    qq_tiles[msb][64:128, mo : mo + QB],
                    start=True,
                    stop=True,
                )
            else:
                k1 = kt_tiles[j1 // 8][0:64, (j1 % 8) * 128 : (j1 % 8 + 1) * 128]
                nc.tensor.matmul(
                    st[:, 512:1024],
                    k1,
                    qv_tiles[msb][0:64, mo : mo + QB],
                    start=True,
                    stop=True,
                )
            pt = pool.tile([128, 1024], bf16, tag=f"pk{m}_{jp}" if pool is parkp else "pt",
                           name=f"pt{m}_{jp}")
            if _dve_pair(jp):
                nc.vector.tensor_scalar(
                    out=pt[:].bitcast(i16),
                    in0=st[:],
                    scalar1=float(SCH_A),
                    scalar2=float(SCH_B),
                    op0=MULT,
                    op1=ADD,
                )
            else:
                nc.scalar.activation(pt[:], st[:], EXP, scale=0.125)
            return pt

        def emit_pv(m, jp, pt, ot):
            """PV matmuls for chunk pair jp into the packed accumulator
            (P^T slices stationary, [V|ones] moving, natural [q, h] out)."""
            for t in range(2):
                j = 2 * jp + t
                for qc in range(4):
                    nc.tensor.matmul(
                        ot[:, qc * 65 : qc * 65 + 65],
                        pt[:, t * 512 + qc * 128 : t * 512 + (qc + 1) * 128],
                        v_tiles[j][:, 0:65],
                        start=False,
                        stop=(j == NCH - 1 and qc == 3),
                        skip_group_check=True,
                    )

        # ---------------- projection passes, with early S^T parked ----------
        parked = {}  # (m, jp) -> pt tile

        def emit_pass(kind, sb, fillers):
            w_tiles, dst = (wkk, kt_tiles[sb]) if kind == "kk" else (wqv, qv_tiles[sb])
            pj = ps.tile([128, SB], f32, tag="st", bufs=3, name=f"pj_{kind}{sb}")
            for half in range(SB // 512):
                for c in range(EC):
                    nc.tensor.matmul(
                        pj[:, half * 512 : (half + 1) * 512],
                        w_tiles[c][:],
                        xt_blk[c][:, sb * SB + half * 512 : sb * SB + (half + 1) * 512],
                        start=(c == 0),
                        stop=(c == EC - 1),
                    )
                # interleave a couple of parked score jobs per half
                for _ in range(2):
                    if fillers:
                        fm, fjp = fillers.pop(0)
                        parked[(fm, fjp)] = emit_st_pair(fm, fjp, parkp)
            if kind == "kk":
                nc.scalar.activation(dst[:], pj[:], COPY)
            else:
                nc.vector.tensor_copy(dst[:], pj[:])
            if kind == "qv":
                # V natural tiles from V^T rows; Q^T dup for the odd row tiles
                nc.sync.dma_start(
                    out=qq_tiles[sb][64:128, :], in_=qv_tiles[sb][0:64, :]
                )
                for u in range(SB // 128):
                    j = sb * (SB // 128) + u
                    pv = ps.tile([128, 64], bf16, tag="ot", bufs=2, name=f"pv{j}")
                    nc.tensor.transpose(
                        pv[:],
                        qv_tiles[sb][64:128, u * 128 : (u + 1) * 128],
                        eye[64:128, 64:128],
                    )
                    nc.vector.tensor_copy(v_tiles[j][:, 0:64], pv[:])

        # early jobs become available progressively:
        #   after kk0+qv0: (m0|m1, j in sb0); after kk1+qv1: (m0..m3, j sb0/sb1)
        fill01 = [(0, jp) for jp in range(4)] + [(1, jp) for jp in range(4)]
        fill23 = [(0, jp) for jp in range(4, 8)] + [(1, jp) for jp in range(4, 8)]
        emit_pass("kk", 0, [])
        emit_pass("qv", 0, [])
        emit_pass("kk", 1, fill01)
        emit_pass("qv", 1, fill01)
        emit_pass("kk", 2, fill23)
        emit_pass("qv", 2, fill23)
        emit_pass("kk", 3, fill23)
        emit_pass("qv", 3, fill23)
        for fm, fjp in fill01 + fill23:
            parked[(fm, fjp)] = emit_st_pair(fm, fjp, parkp)

        # ---------------- attention main loop ----------------
        for m in range(NQB):
            ot = ps.tile([128, 260], f32, tag="ot", bufs=2, name=f"ot{m}")
            # zero the packed accumulator with a K=1 matmul (start=True marks
            # the whole bank's zero region; only this matmul uses start)
            nc.tensor.matmul(
                ot[:, 0:260], zb[0:1, 0:128], zb[0:1, 0:260], start=True, stop=True,
                skip_group_check=True,
            )
            for jp in range(NPAIR):
                pt = parked.pop((m, jp), None)
                if pt is None:
                    pt = emit_st_pair(m, jp, ptp)
                emit_pv(m, jp, pt, ot)
            # epilogue: strided reciprocal of the 4 den columns, 4 scaled copies
            rc = eop.tile([128, 4], f32, tag="rc", name=f"rc{m}")
            nc.vector.reciprocal(rc[:], ot[:, 64:260:65])
            ob = eop.tile([128, 256], f32, tag="ob", name=f"ob{m}")
            for qc in range(4):
                nc.scalar.activation(
                    ob[:, qc * 64 : (qc + 1) * 64],
                    ot[:, qc * 65 : qc * 65 + 64],
                    COPY,
                    scale=rc[:, qc : qc + 1],
                )
            nc.sync.dma_start(
                out=out_d[m * QB : (m + 1) * QB, :].rearrange(
                    "(t p) h -> p t h", p=128
                ),
                in_=ob[:].rearrange("p (t h) -> p t h", h=H),
            )


def build_nc(iters=1):
    key = ("nc", iters)
    if key in _cache:
        return _cache[key]

    nc = bacc.Bacc("TRN2", target_bir_lowering=False, debug=False, num_devices=B)

    x_d = nc.dram_tensor("x", [S, E], bf16, kind="ExternalInput")
    wqv_d = nc.dram_tensor("wqv", [E, 128], bf16, kind="ExternalInput")
    wkk_d = nc.dram_tensor("wkk", [E, 128], bf16, kind="ExternalInput")
    eye_d = nc.dram_tensor("eye", [128, 128], bf16, kind="ExternalInput")
    ones_d = nc.dram_tensor("ones", [128, 1], bf16, kind="ExternalInput")
    zeros_d = nc.dram_tensor("zeros", [1, 384], bf16, kind="ExternalInput")
    out_d = nc.dram_tensor("out", [S, H], f32, kind="ExternalOutput")

    EC = E // 128

    with tile.TileContext(nc) as tc:
        with (
            tc.tile_pool(name="const", bufs=1) as cp,
            tc.tile_pool(name="persist", bufs=1) as pp,
            tc.tile_pool(name="ps", bufs=1, space="PSUM") as ps,
        ):
            eye = cp.tile([128, 128], bf16, tag="eye")
            nc.sync.dma_start(out=eye[:], in_=eye_d[:])
            wqv = []
            wkk = []
            for c in range(EC):
                wq_t = cp.tile([128, 128], bf16, tag=f"wqv{c}")
                wk_t = cp.tile([128, 128], bf16, tag=f"wkk{c}")
                nc.sync.dma_start(out=wq_t[:], in_=wqv_d[c * 128 : (c + 1) * 128, :])
                nc.sync.dma_start(out=wk_t[:], in_=wkk_d[c * 128 : (c + 1) * 128, :])
                wqv.append(wq_t)
                wkk.append(wk_t)

            consts = (eye, wqv, wkk, x_d, ones_d, zeros_d, out_d)
            for _ in range(iters):
                _emit_iteration(nc, tc, ps, pp, consts)

    nc.compile()
    _cache[key] = nc
    return nc


def make_in_maps(x, Wk, Wq, Wv):
    import ml_dtypes

    bf = ml_dtypes.bfloat16
    wqv = np.concatenate([Wq, Wv], axis=1).astype(bf)
    wkk = np.concatenate([Wk, Wk], axis=1).astype(bf)
    eye = np.eye(128, dtype=bf)
    x = np.asarray(x, np.float32).astype(bf)
    return [
        {
            "x": np.ascontiguousarray(x[i]),
            "wqv": wqv,
            "wkk": wkk,
            "eye": eye,
            "ones": np.ones((128, 1), dtype=bf),
            "zeros": np.zeros((1, 384), dtype=bf),
        }
        for i in range(B)
    ]


def kernel(x, Wk, Wq, Wv):
    nc = build_nc()
    in_maps = make_in_maps(np.asarray(x), np.asarray(Wk), np.asarray(Wq), np.asarray(Wv))
    res = run_bass_kernel_spmd(nc, in_maps, core_ids=list(range(B)))
    return np.stack([res.results[i]["out"] for i in range(B)], axis=0)


# revision 6
# speedup vs baseline: 25.2325x; 25.2325x over previous
"""Single-head attention (batch 8, seq 4096, embed 1024, head 64) on 8 TRN2
NeuronCores, data-parallel over batch (one batch element per core).

v2 design (all matmuls bf16, fp32 PSUM):
  1. x^T via DMA-transpose split per (e-chunk, s-block) so projections start
     after the first 2MB lands rather than all 8MB.
  2. Projections: [Wq|Wv] pass -> qv tiles (Q^T rows 0:64, V^T rows 64:128),
     [Wk|Wk] pass -> kt tiles (K^T duplicated in both partition halves).
     Q^T duplicated to partitions 64:128 of qq tiles via SBUF->SBUF DMA.
     V^T PE-transposed to natural [128k, 65] tiles with a ones column
     (softmax denominator rides the PV matmul).
  3. S^T row-tiled: even k-chunks matmul in PE rows 0:64, odd chunks in rows
     64:128 (tile_position row groups) - the two N=512 matmuls run
     concurrently, halving score-matmul time. Each pair writes one
     [128, 1024] PSUM tile (bank-aligned halves).
  4. exp split across two engines: ScalarE true exp (table spline) and DVE
     Schraudolph (one tensor_scalar: i16 = s*23.0812 + B, bitcast bf16),
     interleaved per chunk-pair; bias tuned for zero mean error.
  5. PV with P^T stationary (FWL weight load) and [V|ones] as the N=65
     moving operand: out[q,h] lands in natural layout with the softmax
     denominator in column 64 - no transpose epilogue. Four 65-wide
     accumulators pack into one PSUM bank, pre-zeroed by a K=1 matmul so
     all PV matmuls accumulate with start=False (avoids the 2KB
     zero-region hazard).
  6. Epilogue: strided reciprocal of the 4 denominator columns (DVE), four
     ScalarE copy-with-scale normalizes, one DMA out per 512-q block.
"""

import numpy as np

import concourse.bass as bass
import concourse.mybir as mybir
import concourse.tile as tile
from concourse import bacc
from concourse.bass_utils import run_bass_kernel_spmd

S = 4096  # sequence length (per core)
E = 1024  # embed dim
H = 64  # head size
B = 8  # batch == number of cores

SB = 1024  # projection s-block
NSB = S // SB
QB = 512  # attention q-block
NQB = S // QB
CH = 128  # k chunk
NCH = S // CH
NPAIR = NCH // 2  # chunk pairs per q-block

f32 = mybir.dt.float32
bf16 = mybir.dt.bfloat16
i16 = mybir.dt.int16
EXP = mybir.ActivationFunctionType.Exp
COPY = mybir.ActivationFunctionType.Copy
MULT = mybir.AluOpType.mult
ADD = mybir.AluOpType.add

_cache = {}

# --- tuning flags -----------------------------------------------------------
ROWTILE = True  # concurrent row-tiled S^T pairs
PV_STAT = True  # P^T-stationary PV (natural output) vs V-stationary
N_DVE = 7  # of every 16 chunk-pairs, how many exp'd by DVE Schraudolph
SCH_A = 128.0 * np.log2(np.e) / 8.0  # 23.0812... (the /8 score scale folded in)
SCH_B = 16256.0 - 7.3  # zero-mean bias; +0.5 if hw truncates on int convert


def _dve_pair(jp):
    # Bresenham-interleave N_DVE of every 16 pairs onto the DVE
    return ((jp * N_DVE) % NPAIR) < N_DVE


def _emit_iteration(nc, tc, ps, pp, consts):
    eye, eyef, wqv, wkk, x_d, ones_d, zeros_d, out_d = consts
    EC = E // 128

    qv_tiles = []  # [128, SB]: rows 0:64 Q^T, rows 64:128 V^T
    kt_tiles = []  # [128, SB]: K^T duplicated in both halves
    qq_tiles = []  # [128, SB]: rows 64:128 = Q^T copy (DMA)
    for sb in range(NSB):
        qv_tiles.append(pp.tile([128, SB], bf16, tag=f"qv{sb}", name=f"qv{sb}"))
        kt_tiles.append(pp.tile([128, SB], bf16, tag=f"kt{sb}", name=f"kt{sb}"))
        qq_tiles.append(pp.tile([128, SB], bf16, tag=f"qq{sb}", name=f"qq{sb}"))
    v_tiles = []  # [128, 65] V natural + ones column
    for j in range(NCH):
        v_tiles.append(pp.tile([128, 65], bf16, tag=f"v{j}", name=f"v{j}"))
    zb = pp.tile([1, 384], bf16, tag="zb", name="zb")
    nc.sync.dma_start(out=zb[:], in_=zeros_d[:])
    for j in range(NCH):
        nc.sync.dma_start(out=v_tiles[j][:, 64:65], in_=ones_d[:])

    with (
        tc.tile_pool(name="xt", bufs=1) as xtp,
        tc.tile_pool(name="pt", bufs=4) as ptp,
        tc.tile_pool(name="park", bufs=1) as parkp,
        tc.tile_pool(name="eo", bufs=2) as eop,
    ):
        xt_blk = []
        for c in range(EC):
            xt_c = xtp.tile([128, S], bf16, tag=f"xt{c}", name=f"xt{c}")
            xt_blk.append(xt_c)
        # split transposed loads: s-block-major so early passes unblock first
        for sb in range(NSB):
            for c in range(EC):
                nc.sync.dma_start_transpose(
                    xt_blk[c][:, sb * SB : (sb + 1) * SB],
                    x_d[sb * SB : (sb + 1) * SB, c * 128 : (c + 1) * 128],
                )

        # ---------------- S^T + exp helpers ----------------
        def emit_st_pair(m, jp, pool):
            """Row-tiled score pair for q-block m, chunk pair jp -> exp'd P^T
            pair tile [128, 1024] bf16 (cols 0:512 chunk 2jp, 512:1024 chunk
            2jp+1). Returns the SBUF tile."""
            msb, mo = m // 2, (m % 2) * QB
            j0, j1 = 2 * jp, 2 * jp + 1
            st = ps.tile([128, 1024], f32, tag="st", bufs=3, name=f"st{m}_{jp}")
            k0 = kt_tiles[j0 // 8][0:64, (j0 % 8) * 128 : (j0 % 8 + 1) * 128]
            nc.tensor.matmul(
                st[:, 0:512],
                k0,
                qv_tiles[msb][0:64, mo : mo + QB],
                start=True,
                stop=True,
            )
            if ROWTILE:
                k1 = kt_tiles[j1 // 8][64:128, (j1 % 8) * 128 : (j1 % 8 + 1) * 128]
                nc.tensor.matmul(
                    st[:, 512:1024],
                    k1,
                    qq_tiles[msb][64:128, mo : mo + QB],
                    start=True,
                    stop=True,
                )
            else:
                k1 = kt_tiles[j1 // 8][0:64, (j1 % 8) * 128 : (j1 % 8 + 1) * 128]
                nc.tensor.matmul(
                    st[:, 512:1024],
                    k1,
                    qv_tiles[msb][0:64, mo : mo + QB],
                    start=True,
                    stop=True,
                )
            pt = pool.tile([128, 1024], bf16, tag=f"pk{m}_{jp}" if pool is parkp else "pt",
                           name=f"pt{m}_{jp}")
            if _dve_pair(jp):
                nc.vector.tensor_scalar(
                    out=pt[:].bitcast(i16),
                    in0=st[:],
                    scalar1=float(SCH_A),
                    scalar2=float(SCH_B),
                    op0=MULT,
                    op1=ADD,
                )
            else:
                nc.scalar.activation(pt[:], st[:], EXP, scale=0.125)
            return pt

        def emit_pv(m, jp, pt, ot):
            """PV matmuls for chunk pair jp into the packed accumulator
            (P^T slices stationary, [V|ones] moving, natural [q, h] out)."""
            if not PV_STAT:
                for t in range(2):
                    j = 2 * jp + t
                    nc.tensor.matmul(
                        ot[0:65, :],
                        v_tiles[j][:, 0:65],
                        pt[:, t * 512 : (t + 1) * 512],
                        start=(j == 0),
                        stop=(j == NCH - 1),
                    )
                return
            for t in range(2):
                j = 2 * jp + t
                for qc in range(4):
                    nc.tensor.matmul(
                        ot[:, qc * 65 : qc * 65 + 65],
                        pt[:, t * 512 + qc * 128 : t * 512 + (qc + 1) * 128],
                        v_tiles[j][:, 0:65],
                        start=False,
                        stop=(j == NCH - 1 and qc == 3),
                        skip_group_check=True,
                    )

        # ---------------- projection passes, with early S^T parked ----------
        parked = {}  # (m, jp) -> pt tile

        def emit_pass(kind, sb, fillers):
            w_tiles, dst = (wkk, kt_tiles[sb]) if kind == "kk" else (wqv, qv_tiles[sb])
            pj = ps.tile([128, SB], f32, tag="st", bufs=3, name=f"pj_{kind}{sb}")
            for half in range(SB // 512):
                for c in range(EC):
                    nc.tensor.matmul(
                        pj[:, half * 512 : (half + 1) * 512],
                        w_tiles[c][:],
                        xt_blk[c][:, sb * SB + half * 512 : sb * SB + (half + 1) * 512],
                        start=(c == 0),
                        stop=(c == EC - 1),
                    )
                # interleave a couple of parked score jobs per half
                for _ in range(2):
                    if fillers:
                        fm, fjp = fillers.pop(0)
                        parked[(fm, fjp)] = emit_st_pair(fm, fjp, parkp)
            if kind == "kk":
                nc.scalar.activation(dst[:], pj[:], COPY)
            else:
                nc.vector.tensor_copy(dst[:], pj[:])
            if kind == "qv":
                # V natural tiles from V^T rows; Q^T dup for the odd row tiles
                nc.sync.dma_start(
                    out=qq_tiles[sb][64:128, :], in_=qv_tiles[sb][0:64, :]
                )
                for u in range(SB // 128):
                    j = sb * (SB // 128) + u
                    pv = ps.tile([128, 64], bf16, tag="ot", bufs=2, name=f"pv{j}")
                    nc.tensor.transpose(
                        pv[:],
                        qv_tiles[sb][64:128, u * 128 : (u + 1) * 128],
                        eye[64:128, 64:128],
                    )
                    nc.vector.tensor_copy(v_tiles[j][:, 0:64], pv[:])

        # early jobs become available progressively:
        #   after kk0+qv0: (m0|m1, j in sb0); after kk1+qv1: (m0..m3, j sb0/sb1)
        fill01 = [(0, jp) for jp in range(4)] + [(1, jp) for jp in range(4)]
        fill23 = [(0, jp) for jp in range(4, 8)] + [(1, jp) for jp in range(4, 8)]
        emit_pass("kk", 0, [])
        emit_pass("qv", 0, [])
        emit_pass("kk", 1, fill01)
        emit_pass("qv", 1, fill01)
        emit_pass("kk", 2, fill23)
        emit_pass("qv", 2, fill23)
        emit_pass("kk", 3, fill23)
        emit_pass("qv", 3, fill23)
        for fm, fjp in fill01 + fill23:
            parked[(fm, fjp)] = emit_st_pair(fm, fjp, parkp)

        # ---------------- attention main loop ----------------
        for m in range(NQB):
            if PV_STAT:
                ot = ps.tile([128, 260], f32, tag="ot", bufs=2, name=f"ot{m}")
                # zero the packed accumulator with a K=1 matmul (start=True
                # marks the whole bank's zero region; only this uses start)
                nc.tensor.matmul(
                    ot[:, 0:260], zb[0:1, 0:128], zb[0:1, 0:260], start=True,
                    stop=True, skip_group_check=True,
                )
            else:
                ot = ps.tile([128, 512], f32, tag="ot", bufs=2, name=f"ot{m}")
            for jp in range(NPAIR):
                pt = parked.pop((m, jp), None)
                if pt is None:
                    pt = emit_st_pair(m, jp, ptp)
                emit_pv(m, jp, pt, ot)
            if PV_STAT:
                # epilogue: strided reciprocal of the 4 den cols, scaled copies
                rc = eop.tile([128, 4], f32, tag="rc", name=f"rc{m}")
                nc.vector.reciprocal(rc[:], ot[:, 64:260:65])
                ob = eop.tile([128, 256], f32, tag="ob", name=f"ob{m}")
                for qc in range(4):
                    nc.scalar.activation(
                        ob[:, qc * 64 : (qc + 1) * 64],
                        ot[:, qc * 65 : qc * 65 + 64],
                        COPY,
                        scale=rc[:, qc : qc + 1],
                    )
                nc.sync.dma_start(
                    out=out_d[m * QB : (m + 1) * QB, :].rearrange(
                        "(t p) h -> p t h", p=128
                    ),
                    in_=ob[:].rearrange("p (t h) -> p t h", h=H),
                )
            else:
                # O^T epilogue: transpose numerator blocks + den row, divide
                nt = QB // 128
                ots = eop.tile([96, QB], f32, tag="ots", name=f"ots{m}")
                nc.vector.tensor_copy(ots[0:65, :], ot[0:65, :])
                tp = ps.tile([128, nt * H + nt * 32], f32, tag="st", bufs=3,
                             name=f"tp{m}")
                for t in range(nt):
                    nc.tensor.transpose(
                        tp[:, t * H : (t + 1) * H],
                        ots[0:64, t * 128 : (t + 1) * 128],
                        eyef[0:64, :],
                    )
                    nc.tensor.transpose(
                        tp[:, nt * H + t * 32 : nt * H + (t + 1) * 32],
                        ots[64:96, t * 128 : (t + 1) * 128],
                        eyef[64:96, 0:32],
                    )
                rc = eop.tile([128, nt], f32, tag="rc", name=f"rc{m}")
                d0 = nt * H
                nc.vector.reciprocal(rc[:], tp[:, d0 : d0 + 32 * (nt - 1) + 1 : 32])
                ob = eop.tile([128, nt * H], f32, tag="ob", name=f"ob{m}")
                for t in range(nt):
                    nc.vector.tensor_scalar_mul(
                        ob[:, t * H : (t + 1) * H],
                        tp[:, t * H : (t + 1) * H],
                        rc[:, t : t + 1],
                    )
                nc.sync.dma_start(
                    out=out_d[m * QB : (m + 1) * QB, :].rearrange(
                        "(t p) h -> p t h", p=128
                    ),
                    in_=ob[:].rearrange("p (t h) -> p t h", h=H),
                )


def build_nc(iters=1):
    key = ("nc", iters)
    if key in _cache:
        return _cache[key]

    nc = bacc.Bacc("TRN2", target_bir_lowering=False, debug=False, num_devices=B)

    x_d = nc.dram_tensor("x", [S, E], bf16, kind="ExternalInput")
    wqv_d = nc.dram_tensor("wqv", [E, 128], bf16, kind="ExternalInput")
    wkk_d = nc.dram_tensor("wkk", [E, 128], bf16, kind="ExternalInput")
    eye_d = nc.dram_tensor("eye", [128, 128], bf16, kind="ExternalInput")
    eyef_d = nc.dram_tensor("eyef", [128, 64], f32, kind="ExternalInput")
    ones_d = nc.dram_tensor("ones", [128, 1], bf16, kind="ExternalInput")
    zeros_d = nc.dram_tensor("zeros", [1, 384], bf16, kind="ExternalInput")
    out_d = nc.dram_tensor("out", [S, H], f32, kind="ExternalOutput")

    EC = E // 128

    with tile.TileContext(nc) as tc:
        with (
            tc.tile_pool(name="const", bufs=1) as cp,
            tc.tile_pool(name="persist", bufs=1) as pp,
            tc.tile_pool(name="ps", bufs=1, space="PSUM") as ps,
        ):
            eye = cp.tile([128, 128], bf16, tag="eye")
            nc.sync.dma_start(out=eye[:], in_=eye_d[:])
            eyef = cp.tile([128, 64], f32, tag="eyef")
            nc.sync.dma_start(out=eyef[:], in_=eyef_d[:])
            wqv = []
            wkk = []
            for c in range(EC):
                wq_t = cp.tile([128, 128], bf16, tag=f"wqv{c}")
                wk_t = cp.tile([128, 128], bf16, tag=f"wkk{c}")
                nc.sync.dma_start(out=wq_t[:], in_=wqv_d[c * 128 : (c + 1) * 128, :])
                nc.sync.dma_start(out=wk_t[:], in_=wkk_d[c * 128 : (c + 1) * 128, :])
                wqv.append(wq_t)
                wkk.append(wk_t)

            consts = (eye, eyef, wqv, wkk, x_d, ones_d, zeros_d, out_d)
            for _ in range(iters):
                _emit_iteration(nc, tc, ps, pp, consts)

    nc.compile()
    _cache[key] = nc
    return nc


def _eyef():
    e = np.zeros((128, 64), dtype=np.float32)
    e[0:64, 0:64] = np.eye(64)
    e[64:96, 0:32] = np.eye(32)
    return e


def make_in_maps(x, Wk, Wq, Wv):
    import ml_dtypes

    bf = ml_dtypes.bfloat16
    wqv = np.concatenate([Wq, Wv], axis=1).astype(bf)
    wkk = np.concatenate([Wk, Wk], axis=1).astype(bf)
    eye = np.eye(128, dtype=bf)
    x = np.asarray(x, np.float32).astype(bf)
    return [
        {
            "x": np.ascontiguousarray(x[i]),
            "wqv": wqv,
            "wkk": wkk,
            "eye": eye,
            "ones": np.ones((128, 1), dtype=bf),
            "zeros": np.zeros((1, 384), dtype=bf),
            "eyef": _eyef(),
        }
        for i in range(B)
    ]


def kernel(x, Wk, Wq, Wv):
    nc = build_nc()
    in_maps = make_in_maps(np.asarray(x), np.asarray(Wk), np.asarray(Wq), np.asarray(Wv))
    res = run_bass_kernel_spmd(nc, in_maps, core_ids=list(range(B)))
    return np.stack([res.results[i]["out"] for i in range(B)], axis=0)


# revision 7
# speedup vs baseline: 76.9342x; 3.0490x over previous
"""Single-head attention (batch 8, seq 4096, embed 1024, head 64) on 8 TRN2
NeuronCores, data-parallel over batch (one batch element per core).

v2 design (all matmuls bf16, fp32 PSUM):
  1. x^T via DMA-transpose split per (e-chunk, s-block) so projections start
     after the first 2MB lands rather than all 8MB.
  2. Projections: [Wq|Wv] pass -> qv tiles (Q^T rows 0:64, V^T rows 64:128),
     [Wk|Wk] pass -> kt tiles (K^T duplicated in both partition halves).
     Q^T duplicated to partitions 64:128 of qq tiles via SBUF->SBUF DMA.
     V^T PE-transposed to natural [128k, 65] tiles with a ones column
     (softmax denominator rides the PV matmul).
  3. S^T row-tiled: even k-chunks matmul in PE rows 0:64, odd chunks in rows
     64:128 (tile_position row groups) - the two N=512 matmuls run
     concurrently, halving score-matmul time. Each pair writes one
     [128, 1024] PSUM tile (bank-aligned halves).
  4. exp split across two engines: ScalarE true exp (table spline) and DVE
     Schraudolph (one tensor_scalar: i16 = s*23.0812 + B, bitcast bf16),
     interleaved per chunk-pair; bias tuned for zero mean error.
  5. PV with P^T stationary (FWL weight load) and [V|ones] as the N=65
     moving operand: out[q,h] lands in natural layout with the softmax
     denominator in column 64 - no transpose epilogue. Four 65-wide
     accumulators pack into one PSUM bank, pre-zeroed by a K=1 matmul so
     all PV matmuls accumulate with start=False (avoids the 2KB
     zero-region hazard).
  6. Epilogue: strided reciprocal of the 4 denominator columns (DVE), four
     ScalarE copy-with-scale normalizes, one DMA out per 512-q block.
"""

import numpy as np

import concourse.bass as bass
import concourse.mybir as mybir
import concourse.tile as tile
from concourse import bacc
from concourse.bass_utils import run_bass_kernel_spmd

S = 4096  # sequence length (per core)
E = 1024  # embed dim
H = 64  # head size
B = 8  # batch == number of cores

SB = 1024  # projection s-block
NSB = S // SB
QB = 512  # attention q-block
NQB = S // QB
CH = 128  # k chunk
NCH = S // CH
NPAIR = NCH // 2  # chunk pairs per q-block

f32 = mybir.dt.float32
bf16 = mybir.dt.bfloat16
i16 = mybir.dt.int16
EXP = mybir.ActivationFunctionType.Exp
COPY = mybir.ActivationFunctionType.Copy
MULT = mybir.AluOpType.mult
ADD = mybir.AluOpType.add

_cache = {}

# --- tuning flags -----------------------------------------------------------
ROWTILE = True  # concurrent row-tiled S^T pairs
PV_STAT = True  # P^T-stationary PV (natural output) vs V-stationary
N_DVE = 7  # of every 16 chunk-pairs, how many exp'd by DVE Schraudolph
SCH_A = 128.0 * np.log2(np.e) / 8.0  # 23.0812... (the /8 score scale folded in)
SCH_B = 16256.0 - 7.3  # zero-mean bias; +0.5 if hw truncates on int convert


def _dve_pair(jp):
    # Bresenham-interleave N_DVE of every 16 pairs onto the DVE
    return ((jp * N_DVE) % NPAIR) < N_DVE


def _emit_iteration(nc, tc, ps, pp, consts):
    eye, eyef, wqv, wkk, x_d, ones_d, zeros_d, out_d = consts
    EC = E // 128

    qv_tiles = []  # [128, SB]: rows 0:64 Q^T, rows 64:128 V^T
    kt_tiles = []  # [128, SB]: K^T duplicated in both halves
    qq_tiles = []  # [128, SB]: rows 64:128 = Q^T copy (DMA)
    for sb in range(NSB):
        qv_tiles.append(pp.tile([128, SB], bf16, tag=f"qv{sb}", name=f"qv{sb}"))
        kt_tiles.append(pp.tile([128, SB], bf16, tag=f"kt{sb}", name=f"kt{sb}"))
        qq_tiles.append(pp.tile([128, SB], bf16, tag=f"qq{sb}", name=f"qq{sb}"))
    v_tiles = []  # [128, 65] V natural + ones column
    for j in range(NCH):
        v_tiles.append(pp.tile([128, 65], bf16, tag=f"v{j}", name=f"v{j}"))
    zb = pp.tile([1, 384], bf16, tag="zb", name="zb")
    nc.sync.dma_start(out=zb[:], in_=zeros_d[:])
    for j in range(NCH):
        nc.sync.dma_start(out=v_tiles[j][:, 64:65], in_=ones_d[:])

    with (
        tc.tile_pool(name="xt", bufs=1) as xtp,
        tc.tile_pool(name="pt", bufs=4) as ptp,
        tc.tile_pool(name="park", bufs=1) as parkp,
        tc.tile_pool(name="eo", bufs=2) as eop,
    ):
        xt_blk = []
        for c in range(EC):
            xt_c = xtp.tile([128, S], bf16, tag=f"xt{c}", name=f"xt{c}")
            xt_blk.append(xt_c)
        # split transposed loads: s-block-major so early passes unblock first
        for sb in range(NSB):
            for c in range(EC):
                nc.sync.dma_start_transpose(
                    xt_blk[c][:, sb * SB : (sb + 1) * SB],
                    x_d[sb * SB : (sb + 1) * SB, c * 128 : (c + 1) * 128],
                )

        # ---------------- S^T + exp helpers ----------------
        def emit_st_pair(m, jp, pool):
            """Row-tiled score pair for q-block m, chunk pair jp -> exp'd P^T
            pair tile [128, 1024] bf16 (cols 0:512 chunk 2jp, 512:1024 chunk
            2jp+1). Returns the SBUF tile."""
            msb, mo = m // 2, (m % 2) * QB
            j0, j1 = 2 * jp, 2 * jp + 1
            st = ps.tile([128, 1024], f32, tag="st", bufs=3, name=f"st{m}_{jp}")
            k0 = kt_tiles[j0 // 8][0:64, (j0 % 8) * 128 : (j0 % 8 + 1) * 128]
            nc.tensor.matmul(
                st[:, 0:512],
                k0,
                qv_tiles[msb][0:64, mo : mo + QB],
                start=True,
                stop=True,
            )
            if ROWTILE:
                k1 = kt_tiles[j1 // 8][64:128, (j1 % 8) * 128 : (j1 % 8 + 1) * 128]
                nc.tensor.matmul(
                    st[:, 512:1024],
                    k1,
                    qq_tiles[msb][64:128, mo : mo + QB],
                    start=True,
                    stop=True,
                )
            else:
                k1 = kt_tiles[j1 // 8][0:64, (j1 % 8) * 128 : (j1 % 8 + 1) * 128]
                nc.tensor.matmul(
                    st[:, 512:1024],
                    k1,
                    qv_tiles[msb][0:64, mo : mo + QB],
                    start=True,
                    stop=True,
                )
            pt = pool.tile([128, 1024], bf16, tag=f"pk{m}_{jp}" if pool is parkp else "pt",
                           name=f"pt{m}_{jp}")
            if _dve_pair(jp):
                nc.vector.tensor_scalar(
                    out=pt[:].bitcast(i16),
                    in0=st[:],
                    scalar1=float(SCH_A),
                    scalar2=float(SCH_B),
                    op0=MULT,
                    op1=ADD,
                )
            else:
                nc.scalar.activation(pt[:], st[:], EXP, scale=0.125)
            return pt

        def emit_pv(m, jp, pt, ot):
            """PV matmuls for chunk pair jp into the packed accumulator
            (P^T slices stationary, [V|ones] moving, natural [q, h] out)."""
            if not PV_STAT:
                for t in range(2):
                    j = 2 * jp + t
                    nc.tensor.matmul(
                        ot[0:65, :],
                        v_tiles[j][:, 0:65],
                        pt[:, t * 512 : (t + 1) * 512],
                        start=(j == 0),
                        stop=(j == NCH - 1),
                    )
                return
            for t in range(2):
                j = 2 * jp + t
                for qc in range(4):
                    nc.tensor.matmul(
                        ot[:, qc * 65 : qc * 65 + 65],
                        pt[:, t * 512 + qc * 128 : t * 512 + (qc + 1) * 128],
                        v_tiles[j][:, 0:65],
                        start=False,
                        stop=(j == NCH - 1 and qc == 3),
                        skip_group_check=True,
                    )

        # ---------------- projection passes, with early S^T parked ----------
        parked = {}  # (m, jp) -> pt tile

        def emit_pass(kind, sb, fillers):
            w_tiles, dst = (wkk, kt_tiles[sb]) if kind == "kk" else (wqv, qv_tiles[sb])
            pj = ps.tile([128, SB], f32, tag="st", bufs=3, name=f"pj_{kind}{sb}")
            for half in range(SB // 512):
                for c in range(EC):
                    nc.tensor.matmul(
                        pj[:, half * 512 : (half + 1) * 512],
                        w_tiles[c][:],
                        xt_blk[c][:, sb * SB + half * 512 : sb * SB + (half + 1) * 512],
                        start=(c == 0),
                        stop=(c == EC - 1),
                    )
                # interleave a couple of parked score jobs per half
                for _ in range(2):
                    if fillers:
                        fm, fjp = fillers.pop(0)
                        parked[(fm, fjp)] = emit_st_pair(fm, fjp, parkp)
            if kind == "kk":
                nc.scalar.activation(dst[:], pj[:], COPY)
            else:
                nc.vector.tensor_copy(dst[:], pj[:])
            if kind == "qv":
                # V natural tiles from V^T rows; Q^T dup for the odd row tiles
                nc.sync.dma_start(
                    out=qq_tiles[sb][64:128, :], in_=qv_tiles[sb][0:64, :]
                )
                for u in range(SB // 128):
                    j = sb * (SB // 128) + u
                    pv = ps.tile([128, 64], bf16, tag="ot", bufs=2, name=f"pv{j}")
                    nc.tensor.transpose(
                        pv[:],
                        qv_tiles[sb][64:128, u * 128 : (u + 1) * 128],
                        eye[64:128, 64:128],
                    )
                    nc.vector.tensor_copy(v_tiles[j][:, 0:64], pv[:])

        # parking early score jobs into the prologue measured SLOWER in the
        # timeline sim (PSUM tag contention with the projection passes), so
        # the fillers are disabled.
        fill01 = []
        fill23 = []
        emit_pass("kk", 0, [])
        emit_pass("qv", 0, [])
        emit_pass("kk", 1, fill01)
        emit_pass("qv", 1, fill01)
        emit_pass("kk", 2, fill23)
        emit_pass("qv", 2, fill23)
        emit_pass("kk", 3, fill23)
        emit_pass("qv", 3, fill23)
        for fm, fjp in fill01 + fill23:
            parked[(fm, fjp)] = emit_st_pair(fm, fjp, parkp)

        # ---------------- attention main loop ----------------
        for m in range(NQB):
            if PV_STAT:
                ot = ps.tile([128, 260], f32, tag="ot", bufs=2, name=f"ot{m}")
                # zero the packed accumulator with a K=1 matmul (start=True
                # marks the whole bank's zero region; only this uses start)
                nc.tensor.matmul(
                    ot[:, 0:260], zb[0:1, 0:128], zb[0:1, 0:260], start=True,
                    stop=True, skip_group_check=True,
                )
            else:
                ot = ps.tile([128, 512], f32, tag="ot", bufs=2, name=f"ot{m}")
            for jp in range(NPAIR):
                pt = parked.pop((m, jp), None)
                if pt is None:
                    pt = emit_st_pair(m, jp, ptp)
                emit_pv(m, jp, pt, ot)
            if PV_STAT:
                # epilogue: strided reciprocal of the 4 den cols, scaled copies
                rc = eop.tile([128, 4], f32, tag="rc", name=f"rc{m}")
                nc.vector.reciprocal(rc[:], ot[:, 64:260:65])
                ob = eop.tile([128, 256], f32, tag="ob", name=f"ob{m}")
                for qc in range(4):
                    nc.scalar.activation(
                        ob[:, qc * 64 : (qc + 1) * 64],
                        ot[:, qc * 65 : qc * 65 + 64],
                        COPY,
                        scale=rc[:, qc : qc + 1],
                    )
                nc.sync.dma_start(
                    out=out_d[m * QB : (m + 1) * QB, :].rearrange(
                        "(t p) h -> p t h", p=128
                    ),
                    in_=ob[:].rearrange("p (t h) -> p t h", h=H),
                )
            else:
                # O^T epilogue: transpose numerator blocks + den row, divide
                nt = QB // 128
                ots = eop.tile([96, QB], f32, tag="ots", name=f"ots{m}")
                nc.vector.tensor_copy(ots[0:65, :], ot[0:65, :])
                tp = ps.tile([128, nt * H + nt * 32], f32, tag="st", bufs=3,
                             name=f"tp{m}")
                for t in range(nt):
                    nc.tensor.transpose(
                        tp[:, t * H : (t + 1) * H],
                        ots[0:64, t * 128 : (t + 1) * 128],
                        eyef[0:64, :],
                    )
                    nc.tensor.transpose(
                        tp[:, nt * H + t * 32 : nt * H + (t + 1) * 32],
                        ots[64:96, t * 128 : (t + 1) * 128],
                        eyef[64:96, 0:32],
                    )
                rc = eop.tile([128, nt], f32, tag="rc", name=f"rc{m}")
                d0 = nt * H
                nc.vector.reciprocal(rc[:], tp[:, d0 : d0 + 32 * (nt - 1) + 1 : 32])
                ob = eop.tile([128, nt * H], f32, tag="ob", name=f"ob{m}")
                for t in range(nt):
                    nc.vector.tensor_scalar_mul(
                        ob[:, t * H : (t + 1) * H],
                        tp[:, t * H : (t + 1) * H],
                        rc[:, t : t + 1],
                    )
                nc.sync.dma_start(
                    out=out_d[m * QB : (m + 1) * QB, :].rearrange(
                        "(t p) h -> p t h", p=128
                    ),
                    in_=ob[:].rearrange("p (t h) -> p t h", h=H),
                )


def build_nc(iters=1):
    key = ("nc", iters)
    if key in _cache:
        return _cache[key]

    nc = bacc.Bacc("TRN2", target_bir_lowering=False, debug=False, num_devices=B)

    x_d = nc.dram_tensor("x", [S, E], bf16, kind="ExternalInput")
    wqv_d = nc.dram_tensor("wqv", [E, 128], bf16, kind="ExternalInput")
    wkk_d = nc.dram_tensor("wkk", [E, 128], bf16, kind="ExternalInput")
    eye_d = nc.dram_tensor("eye", [128, 128], bf16, kind="ExternalInput")
    eyef_d = nc.dram_tensor("eyef", [128, 64], f32, kind="ExternalInput")
    ones_d = nc.dram_tensor("ones", [128, 1], bf16, kind="ExternalInput")
    zeros_d = nc.dram_tensor("zeros", [1, 384], bf16, kind="ExternalInput")
    out_d = nc.dram_tensor("out", [S, H], f32, kind="ExternalOutput")

    EC = E // 128

    with tile.TileContext(nc) as tc:
        with (
            tc.tile_pool(name="const", bufs=1) as cp,
            tc.tile_pool(name="persist", bufs=1) as pp,
            tc.tile_pool(name="ps", bufs=1, space="PSUM") as ps,
        ):
            eye = cp.tile([128, 128], bf16, tag="eye")
            nc.sync.dma_start(out=eye[:], in_=eye_d[:])
            eyef = cp.tile([128, 64], f32, tag="eyef")
            nc.sync.dma_start(out=eyef[:], in_=eyef_d[:])
            wqv = []
            wkk = []
            for c in range(EC):
                wq_t = cp.tile([128, 128], bf16, tag=f"wqv{c}")
                wk_t = cp.tile([128, 128], bf16, tag=f"wkk{c}")
                nc.sync.dma_start(out=wq_t[:], in_=wqv_d[c * 128 : (c + 1) * 128, :])
                nc.sync.dma_start(out=wk_t[:], in_=wkk_d[c * 128 : (c + 1) * 128, :])
                wqv.append(wq_t)
                wkk.append(wk_t)

            consts = (eye, eyef, wqv, wkk, x_d, ones_d, zeros_d, out_d)
            for _ in range(iters):
                _emit_iteration(nc, tc, ps, pp, consts)

    nc.compile()
    _cache[key] = nc
    return nc


def _eyef():
    e = np.zeros((128, 64), dtype=np.float32)
    e[0:64, 0:64] = np.eye(64)
    e[64:96, 0:32] = np.eye(32)
    return e


def make_in_maps(x, Wk, Wq, Wv):
    import ml_dtypes

    bf = ml_dtypes.bfloat16
    wqv = np.concatenate([Wq, Wv], axis=1).astype(bf)
    wkk = np.concatenate([Wk, Wk], axis=1).astype(bf)
    eye = np.eye(128, dtype=bf)
    x = np.asarray(x, np.float32).astype(bf)
    return [
        {
            "x": np.ascontiguousarray(x[i]),
            "wqv": wqv,
            "wkk": wkk,
            "eye": eye,
            "ones": np.ones((128, 1), dtype=bf),
            "zeros": np.zeros((1, 384), dtype=bf),
            "eyef": _eyef(),
        }
        for i in range(B)
    ]


def kernel(x, Wk, Wq, Wv):
    nc = build_nc()
    in_maps = make_in_maps(np.asarray(x), np.asarray(Wk), np.asarray(Wq), np.asarray(Wv))
    res = run_bass_kernel_spmd(nc, in_maps, core_ids=list(range(B)))
    return np.stack([res.results[i]["out"] for i in range(B)], axis=0)
